# revision 20
# baseline (speedup 1.0000x reference)
"""Causal single-head attention (B=4, T=2048, E=1024, H=128) on 8 NeuronCores.

Sharding: core = (batch b, half c). Both cores of a batch compute q for ALL
T=2048 queries; the KEYS are split between them in interleaved block pairs
(core 0: key blocks {0,1,4,5,8,9,12,13}, core 1: {2,3,6,7,10,11,14,15}).
For query group g (512 queries), key slots 0..2g+1 of each core exactly cover
the causally needed key range [0, 4g+4) disjointly across the pair, so the
per-core work is perfectly balanced and the SPMD program is identical.

To keep the program identical, the host swaps the two 256-column halves of
every 512-column quarter of x^T for c=1 cores, so each core's own key chunk j
always occupies columns [512j, 512j+256). The q projection then produces
queries in the same swapped order; window masks (per-core data) account for
it, and the host swaps the output columns back.

Each core produces PARTIAL attention sums: num^T[H, T] = sum_k exp(s)·v and
den[1, T] = sum_k exp(s); the host combines halves and normalizes:
out = ((num0+num1)/(den0+den1))^T + bv.

Per-core pipeline (fp16 matmul inputs, fp32 PSUM accumulation):
  e-major projections (W stationary per 128-chunk of E): kT,vT [H,1024 own],
  qT [H,2048]; vT --PE transpose--> v natural [keys,H] per 128-key slot;
  per group g: scores^T[keys,512q] = kT_s^T @ qT_g (slots 0..2g+1), window
  masks on the last two slots, exp (ACT, fp16), AV with v stationary
  accumulating num^T, denominator via DVE slot-sums + one ones-row matmul.
Softmax skips max-subtraction: scores ~ N(0,1), exp is safe in fp16 range.
"""

import math

import numpy as np

import concourse.bass as bass
import concourse.tile as tile
from concourse import bacc, mybir
from concourse.bass_utils import run_bass_kernel_spmd
from concourse.masks import make_identity

B, T, E, H = 4, 2048, 1024, 128
NE = E // 128           # 8 contraction chunks
NS = 8                  # own key slots (128 keys each) per core
BIG = 3.0e4             # fp16-safe; exp(score - 3e4) underflows to exactly 0
N_WARM = 8              # PE p-state warmup matmuls while x streams in

F32 = mybir.dt.float32
F16 = mybir.dt.float16

_CACHE: dict = {}


def _build(debug=False):
    nc = bacc.Bacc(None, target_bir_lowering=False)
    xt = nc.dram_tensor("xt", [E, T], F16, kind="ExternalInput")
    wcat = nc.dram_tensor("wcat", [128, 3 * E], F16, kind="ExternalInput")
    bq = nc.dram_tensor("bq", [H, 1], F32, kind="ExternalInput")
    bk = nc.dram_tensor("bk", [H, 1], F32, kind="ExternalInput")
    msk = nc.dram_tensor("msk", [128, 1024], F16, kind="ExternalInput")
    y_num = nc.dram_tensor("y_num", [H, T], F16, kind="ExternalOutput")
    y_den = nc.dram_tensor("y_den", [1, T], F32, kind="ExternalOutput")
    if debug:
        dbg_q = nc.dram_tensor("dbg_q", [H, T], F16, kind="ExternalOutput")
        dbg_k = nc.dram_tensor("dbg_k", [H, NS * 128], F16, kind="ExternalOutput")
        dbg_v = nc.dram_tensor("dbg_v", [H, NS * 128], F16, kind="ExternalOutput")
        dbg_vn = nc.dram_tensor("dbg_vn", [128, NS * 128], F16, kind="ExternalOutput")

    with tile.TileContext(nc) as tc:
        with (
            tc.tile_pool(name="xs", bufs=1) as xs_pool,
            tc.tile_pool(name="wsb", bufs=1) as w_pool,
            tc.tile_pool(name="small", bufs=1) as sm_pool,
            tc.tile_pool(name="qk", bufs=1) as qk_pool,
            tc.tile_pool(name="ex", bufs=10) as ex_pool,
        ):
            # ---- DMA in (ACT HWDGE queue, in consumption order: wk, wv,
            # then x quarter by quarter with wq before quarter 1)
            wsb = w_pool.tile([128, 3 * E], F16)
            nc.scalar.dma_start(wsb[:, 0:E], wcat[:, 0:E])
            nc.scalar.dma_start(wsb[:, E:2 * E], wcat[:, E:2 * E])
            xs = xs_pool.tile([128, NE * T], F16, name="xs", tag="xs")
            for j in range(4):
                for e in range(NE):
                    nc.scalar.dma_start(
                        xs[:, e * T + j * 512: e * T + (j + 1) * 512],
                        xt[e * 128:(e + 1) * 128, j * 512:(j + 1) * 512])
                if j == 0:
                    nc.scalar.dma_start(wsb[:, 2 * E:3 * E], wcat[:, 2 * E:3 * E])
            # small inputs on the SP HWDGE queue
            bq_sb = sm_pool.tile([128, 1], F32, tag="bq")
            bk_sb = sm_pool.tile([128, 1], F32, tag="bk")
            nc.sync.dma_start(bq_sb[:], bq[:])
            nc.sync.dma_start(bk_sb[:], bk[:])
            mask_sb = sm_pool.tile([128, 1024], F16, tag="msk")
            nc.sync.dma_start(mask_sb[:], msk[:])

            ident = sm_pool.tile([128, 128], F16, tag="ident")
            make_identity(nc, ident[:])
            ones = sm_pool.tile([128, 1], F16, tag="ones")
            nc.vector.memset(ones[:], 1.0)
            scratch = sm_pool.tile([128, 512], F16, tag="scratch")
            nc.vector.memset(scratch[:], 0.0)

            kT = qk_pool.tile([128, NS * 128], F16, tag="kT")
            vT = qk_pool.tile([128, NS * 128], F16, tag="vT")
            vnat = qk_pool.tile([128, NS * 128], F16, tag="vnat")
            qT = qk_pool.tile([128, T], F16, tag="qT")

            # ---- quarter-major pipeline: per 512-col quarter j of x, do
            # kv-proj chunk j, q-proj group j, v transposes, then attend(j)
            with (
                tc.tile_pool(name="kvq", bufs=1, space="PSUM") as kvq,
                tc.tile_pool(name="qps", bufs=1, space="PSUM") as qps_pool,
                tc.tile_pool(name="tps", bufs=1, space="PSUM") as tps,
                tc.tile_pool(name="sps", bufs=2, space="PSUM") as sps,
                tc.tile_pool(name="aps", bufs=2, space="PSUM") as aps,
                tc.tile_pool(name="dps", bufs=1, space="PSUM") as dps,
            ):
                # warm the PE p-state while the weights + first x slices
                # stream in (the first real accumulation clears the bank)
                for i in range(N_WARM):
                    wp = qps_pool.tile([128, 512], F32, name=f"warm{i}",
                                       tag="qp")
                    nc.tensor.matmul(wp[:], ident[:], scratch[:],
                                     start=True, stop=True)

                for j in range(4):
                    # k_j | v_j share one PSUM bank: single bank-clearing
                    # start on the first k matmul; v's first matmul writes
                    # fresh via per-element has_written
                    kvp = kvq.tile([128, 512], F32, name=f"kvp{j}", tag="kv")
                    for e in range(NE):
                        nc.tensor.matmul(
                            kvp[:, 0:256], wsb[:, e * 128:(e + 1) * 128],
                            xs[:, e * T + j * 512: e * T + j * 512 + 256],
                            start=(e == 0), stop=(e == NE - 1),
                            skip_group_check=True)
                        nc.tensor.matmul(
                            kvp[:, 256:512], wsb[:, E + e * 128: E + (e + 1) * 128],
                            xs[:, e * T + j * 512: e * T + j * 512 + 256],
                            start=False, stop=(e == NE - 1),
                            skip_group_check=True)
                    qp = qps_pool.tile([128, 512], F32, name=f"qp{j}", tag="qp")
                    for e in range(NE):
                        nc.tensor.matmul(
                            qp[:],
                            wsb[:, 2 * E + e * 128: 2 * E + (e + 1) * 128],
                            xs[:, e * T + j * 512: e * T + (j + 1) * 512],
                            start=(e == 0), stop=(e == NE - 1))
                    nc.scalar.activation(
                        kT[:, j * 256:(j + 1) * 256], kvp[:, 0:256],
                        mybir.ActivationFunctionType.Identity, bias=bk_sb[:])
                    nc.vector.tensor_copy(vT[:, j * 256:(j + 1) * 256],
                                          kvp[:, 256:512])
                    nc.scalar.activation(
                        qT[:, j * 512:(j + 1) * 512], qp[:],
                        mybir.ActivationFunctionType.Identity, bias=bq_sb[:])
                    for s in (2 * j, 2 * j + 1):
                        tp = tps.tile([128, 128], F16)
                        nc.tensor.transpose(
                            tp[:], vT[:, s * 128:(s + 1) * 128], ident[:])
                        nc.vector.tensor_copy(vnat[:, s * 128:(s + 1) * 128],
                                              tp[:])

                    # ---- attention for query group j: slots 0..2j+1,
                    # masked (diagonal-window) slots first, sp/av staggered
                    # so the av matmul never stalls PE on a fresh exp
                    g, L = j, 2 * j + 2
                    slots = [2 * g, 2 * g + 1] + list(range(2 * g))
                    av = aps.tile([128, 512], F32, name=f"av{g}", tag="av")
                    den = dps.tile([1, 512], F32, name=f"den{g}", tag="den")
                    sum_ex = ex_pool.tile([128, 512], F16, name=f"sume{g}",
                                          tag="sumex")
                    exs = []

                    def av_mm(idx):
                        nc.tensor.matmul(
                            av[:],
                            vnat[:, slots[idx] * 128:(slots[idx] + 1) * 128],
                            exs[idx][:], start=(idx == 0), stop=(idx == L - 1))

                    for idx, s in enumerate(slots):
                        sp = sps.tile([128, 512], F32)
                        nc.tensor.matmul(
                            sp[:], kT[:, s * 128:(s + 1) * 128],
                            qT[:, g * 512:(g + 1) * 512], start=True, stop=True)
                        if idx < 2:
                            nc.vector.tensor_add(
                                sp[:], sp[:],
                                mask_sb[:, idx * 512:(idx + 1) * 512])
                        ex = ex_pool.tile([128, 512], F16)
                        nc.scalar.activation(
                            ex[:], sp[:], mybir.ActivationFunctionType.Exp)
                        exs.append(ex)
                        if idx >= 1:
                            av_mm(idx - 1)
                        if idx == 1:
                            nc.vector.tensor_add(sum_ex[:], exs[0][:], exs[1][:])
                        elif idx > 1:
                            nc.vector.tensor_add(sum_ex[:], sum_ex[:], ex[:])
                    av_mm(L - 1)
                    nc.tensor.matmul(den[:], ones[:], sum_ex[:], start=True,
                                     stop=True)
                    av_sb = ex_pool.tile([128, 512], F16, name=f"avsb{g}",
                                         tag="avsb")
                    nc.vector.tensor_copy(av_sb[:], av[:])
                    den_sb = ex_pool.tile([1, 512], F32, name=f"densb{g}",
                                          tag="densb")
                    nc.vector.tensor_copy(den_sb[:], den[:])
                    nc.sync.dma_start(y_num[:, g * 512:(g + 1) * 512], av_sb[:])
                    nc.sync.dma_start(y_den[:, g * 512:(g + 1) * 512], den_sb[:])

                if debug:
                    nc.sync.dma_start(dbg_q[:], qT[:])
                    nc.sync.dma_start(dbg_k[:], kT[:])
                    nc.sync.dma_start(dbg_v[:], vT[:])
                    nc.sync.dma_start(dbg_vn[:], vnat[:])
    nc.compile()
    return nc


def _masks(c: int) -> np.ndarray:
    p = np.arange(128)[:, None]  # key within block (partition)
    q = np.arange(128)[None, :]  # query within block (free)
    tri = np.where(p <= q, 0.0, -BIG).astype(np.float16)
    z = np.zeros((128, 128), np.float16)
    inv = np.full((128, 128), -BIG, np.float16)
    pi = (0, 1, 2, 3) if c == 0 else (2, 3, 0, 1)  # natural q block at quarter
    parts = []
    for r in (2 * c, 2 * c + 1):  # own window key blocks 4g+r
        quarters = [tri if r == pi[cq] else (z if r < pi[cq] else inv)
                    for cq in range(4)]
        parts.append(np.concatenate(quarters, axis=1))
    return np.ascontiguousarray(np.concatenate(parts, axis=1))


def _half_swap(a: np.ndarray) -> np.ndarray:
    # swap the two 256-column halves of every 512-column quarter
    n = a.shape[-1] // 512
    return np.ascontiguousarray(
        a.reshape(*a.shape[:-1], n, 2, 256)[..., ::-1, :].reshape(*a.shape))


def kernel(x, Wq, bq, Wk, bk, Wv, bv):
    x = np.asarray(x, dtype=np.float32)
    Wq = np.asarray(Wq, dtype=np.float32)
    Wk = np.asarray(Wk, dtype=np.float32)
    Wv = np.asarray(Wv, dtype=np.float32)
    bq = np.asarray(bq, dtype=np.float32)
    bk = np.asarray(bk, dtype=np.float32)
    bv = np.asarray(bv, dtype=np.float32)

    if "nc" not in _CACHE:
        _CACHE["nc"] = _build()
    nc = _CACHE["nc"]

    scale = 1.0 / math.sqrt(H)
    # wcat[p, proj*E + e*128 + h] = W[e*128 + p, h], proj order (k, v, q)
    wcat = np.empty((128, 3 * E), dtype=np.float16)
    for pi_, w in enumerate((Wk, Wv, Wq * scale)):
        wcat[:, pi_ * E:(pi_ + 1) * E] = (
            w.astype(np.float16).reshape(NE, 128, H).transpose(1, 0, 2)
            .reshape(128, E))
    bq_s = np.ascontiguousarray((bq * scale).reshape(H, 1))
    bk_r = np.ascontiguousarray(bk.reshape(H, 1))
    masks = {0: _masks(0), 1: _masks(1)}

    xt = np.ascontiguousarray(x.transpose(0, 2, 1).astype(np.float16))  # [B,E,T]
    xt_sw = {0: xt, 1: None}
    in_maps = []
    for core in range(8):
        b, c = divmod(core, 2)
        xtc = xt[b] if c == 0 else _half_swap(xt[b])
        in_maps.append({
            "xt": xtc, "wcat": wcat,
            "bq": bq_s, "bk": bk_r, "msk": masks[c],
        })

    res = run_bass_kernel_spmd(nc, in_maps, core_ids=list(range(8)))
    out = np.empty((B, T, H), dtype=np.float32)
    for b in range(B):
        r0, r1 = res.results[2 * b], res.results[2 * b + 1]
        num = (r0["y_num"].astype(np.float32)
               + _half_swap(r1["y_num"]).astype(np.float32))   # [H, T]
        den = r0["y_den"] + _half_swap(r1["y_den"])            # [1, T]
        out[b] = (num / den).T
    out += bv  # sum_j softmax_ij = 1, so +bv commutes with attention
    return out
